# revision 7
# baseline (speedup 1.0000x reference)
"""DFlashAttention kernel for Trainium2, tensor-parallel across 8 NeuronCores.

Sharding: Megatron-style head parallelism. Core c owns KV head c and Q heads
4c..4c+3 (matches repeat_interleave grouping), i.e. Wq rows [512c, 512c+512),
Wk/Wv rows [128c, 128c+128), Wo columns [512c, 512c+512). Each core computes a
partial output [QL, H]; the host sums the 8 partials (row-parallel Wo).

v2 layout/pipeline notes:
  - activations/weights fed feature-major (host pre-transposes), bf16
  - Q/K kept d-major [HD, seq] f32r for scores^T = K^T(dxk-tile) @ Q(dxq)
  - scores matmul pairs write a 2-bank [128,2,512] PSUM tile; one ACT exp
    covers both halves (halves ACT instruction overhead)
  - softmax denominator: rowsum matmuls packed 4-wide into PE column groups
    via tile_position (4 concurrent M=1 matmuls ~ cost of one), partials at
    partitions 0/32/64/96 summed by a ones4 matmul; reciprocal on DVE;
    denominator broadcast along partitions via PE ones matmul
  - V kept k-major [seq, HD] (PE-transposed after d-major projection), bf16;
    PV accumulates attn^T = V^T @ P^T in PSUM over 32 k-tiles
  - Wo bf16, preloaded at kernel start; Wo matmuls of q-block n interleaved
    into the attention of q-block n+1 so the PE stays fed while ACT does exp
  - RoPE rotate-half swap via DVE partition-offset copies (no DMA);
    target_hidden loads issued on the ACT DMA queue to halve SP pressure
"""

import math
from contextlib import ExitStack

import ml_dtypes
import numpy as np

import concourse.bass as bass
import concourse.bacc as bacc
import concourse.mybir as mybir
import concourse.tile as tile
from concourse.bass_utils import run_bass_kernel_spmd

F32 = mybir.dt.float32
F32R = mybir.dt.float32r
BF16 = mybir.dt.bfloat16
AF = mybir.ActivationFunctionType
ALU = mybir.AluOpType

# Full-problem dims (hardcoded per spec)
B, QL, CTX, H = 1, 2048, 2048, 4096
NH, NKV, HD = 32, 8, 128
NCORES = 8
HPC = NH // NKV  # 4 q-heads per core (one KV head per core)


def build_program(ql=QL, ctx_len=CTX, h=H, trace_sim=False, phases="ABC", body_reps=1):
    """Build the per-core Bass program (SPMD: same program, per-core shards)."""
    s = ql + ctx_len          # total kv length
    et = h // 128             # e-tiles (contraction tiles for projections)
    kt = s // 128             # k-tiles in attention
    QC = 512                  # phase A position-chunk
    nch = ql // QC            # chunks (ctx assumed == ql)
    assert ctx_len == ql, "phase A chunking assumes ctx_len == ql"
    QB = 512                  # phase B q-block
    nqb = ql // QB
    scale = 1.0 / math.sqrt(HD)
    DQ = HPC * HD             # 512: per-core q-head dim
    hot = h // 512            # output-column chunks in Wo stage

    nc = bacc.Bacc("TRN2", target_bir_lowering=False, debug=False)

    def din(name, shape, dt_=F32):
        return nc.dram_tensor(name, shape, dt_, kind="ExternalInput").ap()

    hiddenT = din("hiddenT", [h, ql], BF16)      # hidden_states[0].T
    targetT = din("targetT", [h, ctx_len], BF16)  # target_hidden[0].T
    cosT = din("cosT", [HD, s])            # cos[0].T
    sinT = din("sinT", [HD, s])            # sign-folded sin[0].T
    wqT = din("wqT", [h, DQ], BF16)        # Wq[shard].T
    wkT = din("wkT", [h, HD], BF16)
    wvT = din("wvT", [h, HD], BF16)
    woT = din("woT", [DQ, h], BF16)        # Wo[:, shard].T
    onesb_d = din("ones_bf", [128, 1], BF16)
    onesr_d = din("onesr", [128, 128], F32R)  # rows 0/32/64/96 all-ones
    ident_d = din("ident", [128, 128])
    out_d = nc.dram_tensor("out", [ql, h], F32, kind="ExternalOutput").ap()

    with tile.TileContext(nc, trace_sim=trace_sim) as tc, ExitStack() as ctx:
        persist = ctx.enter_context(tc.tile_pool(name="persist", bufs=1))

        qr_sb = persist.tile([128, HPC, ql], F32R, tag="qr")    # [d, h, q]
        kr_sb = persist.tile([128, s], F32R, tag="kr")          # [d, k]
        v_sb = persist.tile([128, kt, 128], BF16, tag="v")     # [k%128, ktile, d]
        wo_sb = persist.tile([128, HPC, h], BF16, tag="wo")    # [t%128, t//128, o]
        onesb_sb = persist.tile([128, 1], BF16, tag="onesb")
        onesr_sb = persist.tile([128, 128], F32R, tag="onesr")
        ident_sb = persist.tile([128, 128], F32, tag="ident")
        nc.sync.dma_start(onesb_sb[:], onesb_d[:])
        nc.sync.dma_start(onesr_sb[:], onesr_d[:])
        nc.sync.dma_start(ident_sb[:], ident_d[:])

        # ---------------- Phase A: projections + RoPE + V transpose ---------
        for _rep in range(body_reps):
          with (
              tc.tile_pool(name="wpool", bufs=1) as wpool,
              tc.tile_pool(name="apool", bufs=1) as apool,
              tc.tile_pool(name="psA", bufs=8, space=bass.MemorySpace.PSUM) as psA,
          ):
              wq_sb = wpool.tile([128, et, DQ], BF16, tag="wq")   # [e%128, etile, d]
              wk_sb = wpool.tile([128, et, HD], BF16, tag="wk")
              wv_sb = wpool.tile([128, et, HD], BF16, tag="wv")
              wq_r = wqT.rearrange("(e p) d -> p e d", p=128)
              wk_r = wkT.rearrange("(e p) d -> p e d", p=128)
              wv_r = wvT.rearrange("(e p) d -> p e d", p=128)
              wo_r = woT.rearrange("(t p) o -> p t o", p=128)

              def emit_weight_chunk(wc):
                  # interleaved into chunk 0's e-loop: first matmuls start early
                  sl = slice(wc * (et // 4), (wc + 1) * (et // 4))
                  nc.sync.dma_start(wk_sb[:, sl, :], wk_r[:, sl, :])
                  nc.sync.dma_start(wv_sb[:, sl, :], wv_r[:, sl, :])
                  nc.sync.dma_start(wq_sb[:, sl, :], wq_r[:, sl, :])

              def rope(ps_tile, cos_sl, sin_sl, dst, dve_raw=False,
                       act_swp=False):
                  # dst = ps*cos + rot_half(ps)*sin  (sin sign pre-folded)
                  raw = apool.tile([128, QC], F32, tag="rraw", bufs=3)
                  if dve_raw:
                      nc.vector.tensor_copy(raw[:], ps_tile[:])
                  else:
                      nc.scalar.copy(raw[:], ps_tile[:])
                  swp = apool.tile([128, QC], F32, tag="rswp", bufs=3)
                  if act_swp:
                      nc.scalar.copy(swp[0:64, :], raw[64:128, :])
                      nc.scalar.copy(swp[64:128, :], raw[0:64, :])
                  else:
                      nc.vector.tensor_copy(swp[0:64, :], raw[64:128, :])
                      nc.vector.tensor_copy(swp[64:128, :], raw[0:64, :])
                  t1 = apool.tile([128, QC], F32, tag="rt1", bufs=2)
                  nc.vector.tensor_tensor(t1[:], raw[:], cos_sl, ALU.mult)
                  t2 = apool.tile([128, QC], F32, tag="rt2", bufs=2)
                  nc.gpsimd.tensor_tensor(t2[:], swp[:], sin_sl, ALU.mult)
                  nc.vector.tensor_tensor(dst, t1[:], t2[:], ALU.add)

              for c in range(nch):
                  q0 = c * QC
                  cn = apool.tile([128, QC], F32, tag="cn", bufs=1)
                  sn = apool.tile([128, QC], F32, tag="sn", bufs=1)
                  cc = apool.tile([128, QC], F32, tag="cc", bufs=1)
                  sc = apool.tile([128, QC], F32, tag="sc", bufs=1)
                  nc.scalar.dma_start(cn[:], cosT[:, ctx_len + q0:ctx_len + q0 + QC])
                  nc.scalar.dma_start(sn[:], sinT[:, ctx_len + q0:ctx_len + q0 + QC])
                  nc.scalar.dma_start(cc[:], cosT[:, q0:q0 + QC])
                  nc.scalar.dma_start(sc[:], sinT[:, q0:q0 + QC])

                  psq = [
                      psA.tile([128, QC], F32, tag="ps", name=f"psq{i}")
                      for i in range(HPC)
                  ]
                  pskn = psA.tile([128, QC], F32, tag="ps")
                  pskc = psA.tile([128, QC], F32, tag="ps")
                  psvn = psA.tile([128, QC], F32, tag="ps")
                  psvc = psA.tile([128, QC], F32, tag="ps")

                  hid_r = hiddenT.rearrange("(b p) q -> p b q", p=128)
                  tgt_r = targetT.rearrange("(b p) q -> p b q", p=128)
                  for e0 in range(0, et, 2):
                      if c == 0 and e0 % (et // 4) == 0:
                          emit_weight_chunk(e0 // (et // 4))
                      hs = apool.tile([128, 2, QC], BF16, tag="hs", bufs=3)
                      nc.sync.dma_start(
                          hs[:], hid_r[:, e0:e0 + 2, q0:q0 + QC]
                      )
                      ts_ = apool.tile([128, 2, QC], BF16, tag="ts", bufs=3)
                      nc.scalar.dma_start(
                          ts_[:], tgt_r[:, e0:e0 + 2, q0:q0 + QC]
                      )
                      for u in (0, 1):
                          e = e0 + u
                          st = dict(start=(e == 0), stop=(e == et - 1))
                          for hh in range(HPC):
                              nc.tensor.matmul(
                                  psq[hh][:],
                                  wq_sb[:, e, hh * 128:hh * 128 + 128],
                                  hs[:, u, :],
                                  **st,
                              )
                          nc.tensor.matmul(pskn[:], wk_sb[:, e, :], hs[:, u, :], **st)
                          nc.tensor.matmul(psvn[:], wv_sb[:, e, :], hs[:, u, :], **st)
                          nc.tensor.matmul(pskc[:], wk_sb[:, e, :], ts_[:, u, :], **st)
                          nc.tensor.matmul(psvc[:], wv_sb[:, e, :], ts_[:, u, :], **st)

                  # RoPE + V drain. In the last chunk, finish kr/v first so
                  # phase B's first scores aren't gated on the Q ropes.
                  def drain_kv():
                      rope(pskn, cn[:], sn[:],
                           kr_sb[:, ctx_len + q0:ctx_len + q0 + QC], dve_raw=True)
                      rope(pskc, cc[:], sc[:], kr_sb[:, q0:q0 + QC])
                      for vsrc, kbase in ((psvc, q0), (psvn, ctx_len + q0)):
                          vd = apool.tile([128, QC], BF16, tag="vd", bufs=2)
                          nc.scalar.copy(vd[:], vsrc[:])
                          j0 = kbase // 128
                          nc.sync.dma_start_transpose(
                              v_sb[:, j0:j0 + QC // 128, :], vd[:]
                          )

                  def drain_q():
                      for hh in range(HPC):
                          rope(psq[hh], cn[:], sn[:], qr_sb[:, hh, q0:q0 + QC],
                               dve_raw=(hh % 2 == 1), act_swp=True)

                  if c == nch - 1:
                      drain_kv()
                      drain_q()
                  else:
                      drain_q()
                      drain_kv()

                  # wo chunk load on the ACT queue, spread across chunks
                  wsl = slice(c * (HPC // nch) if HPC >= nch else c,
                              (c + 1) * max(HPC // nch, 1))
                  nc.scalar.dma_start(wo_sb[:, wsl, :], wo_r[:, wsl, :])

          # ---------------- Phase B/C: attention + output projection ----------
          with (
              tc.tile_pool(name="bpool", bufs=1) as bpool,
              tc.tile_pool(name="psB", bufs=1, space=bass.MemorySpace.PSUM) as psB,
          ):
              ats = {}

              def emit_head(qb, hh, between=None):
                  qs0 = qb * QB
                  qsl = qr_sb[:, hh, qs0:qs0 + QB]
                  expst = bpool.tile([128, kt, QB], BF16, tag="expst", bufs=2)
                  psat = psB.tile([128, QB], F32, tag="psat", bufs=1)
                  psrs = psB.tile([128, QB], F32, tag="psrs", bufs=1)

                  def emit_scores_pair(g):
                      pss = psB.tile([128, 2, 512], F32, tag="pss", bufs=2)
                      for u in (0, 1):
                          j = 2 * g + u
                          nc.tensor.matmul(
                              pss[:, u, :],
                              kr_sb[:, j * 128:j * 128 + 128],
                              qsl,
                              start=True,
                              stop=True,
                          )
                      nc.scalar.activation(
                          expst[:, 2 * g:2 * g + 2, :], pss[:], AF.Exp, scale=scale
                      )

                  LOOK = 2
                  npair = kt // 2
                  for g in range(min(LOOK, npair)):
                      emit_scores_pair(g)
                  for g in range(npair):
                      if g + LOOK < npair:
                          emit_scores_pair(g + LOOK)
                      for u in (0, 1):
                          j = 2 * g + u
                          nc.tensor.matmul(
                              psat[:],
                              v_sb[:, j, :],
                              expst[:, j, :],
                              start=(j == 0),
                              stop=(j == kt - 1),
                          )
                      if g % 2 == 1:
                          # rowsum quad packed into 4 PE column groups
                          for r in range(4):
                              j = (g // 2) * 4 + r
                              nc.tensor.matmul(
                                  psrs[32 * r:32 * r + 1, :],
                                  onesb_sb[:],
                                  expst[:, j, :],
                                  start=(j < 4),
                                  stop=(j >= kt - 4),
                                  tile_position=(0, 32 * r),
                              )
                      if between is not None:
                          between(g)

                  # denominator: one matmul reduces the 4 col-group partials
                  # AND broadcasts the sum to all 128 partitions (onesr has
                  # all-ones rows at partitions 0/32/64/96)
                  rs_sb = bpool.tile([128, QB], F32R, tag="rs", bufs=2)
                  nc.vector.tensor_copy(rs_sb[:], psrs[:])
                  psd = psB.tile([128, QB], F32, tag="psrs", bufs=1)
                  nc.tensor.matmul(
                      psd[:], onesr_sb[:], rs_sb[:], start=True, stop=True
                  )
                  recd = bpool.tile([128, QB], F32R, tag="recip", bufs=2)
                  with nc.allow_low_precision(
                      reason="f32r reciprocal feeds the normalize multiply"
                  ):
                      nc.vector.reciprocal(recd[:], psd[:])
                  at = bpool.tile([128, QB], BF16, tag="at", bufs=8)
                  nc.vector.tensor_tensor(at[:], psat[:], recd[:], ALU.mult)
                  ats[(qb, hh)] = at

              def emit_c_chain(qb, qs, oc):
                  qs0 = qb * QB
                  pso = psB.tile([128, 512], F32, tag="pso", bufs=2)
                  for t in range(HPC):
                      nc.tensor.matmul(
                          pso[:],
                          ats[(qb, t)][:, qs * 128:qs * 128 + 128],
                          wo_sb[:, t, oc * 512:oc * 512 + 512],
                          start=(t == 0),
                          stop=(t == HPC - 1),
                      )
                  ob = bpool.tile([128, 512], F32, tag="ob", bufs=3)
                  if (qs + oc) % 2 == 0:
                      nc.scalar.copy(ob[:], pso[:])
                  else:
                      nc.vector.tensor_copy(ob[:], pso[:])
                  nc.sync.dma_start(
                      out_d[qs0 + qs * 128:qs0 + qs * 128 + 128,
                            oc * 512:oc * 512 + 512],
                      ob[:],
                  )

              if "B" in phases:
                  for qb in range(nqb):
                      for hh in range(HPC):
                          if qb > 0 and "C" in phases:
                              # one Wo chain after every 2 score pairs: the
                              # pso-bank wait always coincides with ready
                              # attention matmuls
                              emit_head(qb, hh, between=lambda g, _q=qb - 1,
                                        _s=hh: emit_c_chain(_q, _s, g // 2)
                                        if g % 2 == 1 else None)
                          else:
                              emit_head(qb, hh)
                  if "C" in phases:
                      for qs in range(QB // 128):
                          for oc in range(hot):
                              emit_c_chain(nqb - 1, qs, oc)
    return _finish(nc)


def _finish(nc):
    nc.compile()
    return nc


def make_in_maps(hidden_states, target_hidden, cos, sin, Wq, Wk, Wv, Wo):
    hidden_states = np.asarray(hidden_states, dtype=np.float32)
    target_hidden = np.asarray(target_hidden, dtype=np.float32)
    cos = np.asarray(cos, dtype=np.float32)
    sin = np.asarray(sin, dtype=np.float32)
    Wq = np.asarray(Wq, dtype=np.float32)
    Wk = np.asarray(Wk, dtype=np.float32)
    Wv = np.asarray(Wv, dtype=np.float32)
    Wo = np.asarray(Wo, dtype=np.float32)

    bf16 = ml_dtypes.bfloat16
    hT = np.ascontiguousarray(hidden_states[0].T).astype(bf16)
    tT = np.ascontiguousarray(target_hidden[0].T).astype(bf16)
    cT = np.ascontiguousarray(cos[0].T)
    sT = np.ascontiguousarray(sin[0].T).copy()
    sT[:64, :] *= -1.0  # fold rotate_half sign: rot(x)*sin == swap(x)*sT
    ident = np.eye(128, dtype=np.float32)
    onesr = np.zeros((128, 128), dtype=np.float32)
    onesr[[0, 32, 64, 96], :] = 1.0

    in_maps = []
    for c in range(NCORES):
        in_maps.append({
            "hiddenT": hT,
            "targetT": tT,
            "cosT": cT,
            "sinT": sT,
            "wqT": np.ascontiguousarray(Wq[512 * c:512 * c + 512, :].T).astype(bf16),
            "wkT": np.ascontiguousarray(Wk[128 * c:128 * c + 128, :].T).astype(bf16),
            "wvT": np.ascontiguousarray(Wv[128 * c:128 * c + 128, :].T).astype(bf16),
            "woT": np.ascontiguousarray(Wo[:, 512 * c:512 * c + 512].T).astype(bf16),
            "ones_bf": np.ones((128, 1), dtype=bf16),
            "onesr": onesr,
            "ident": ident,
        })
    return in_maps


_CACHE = {}
LAST_EXEC_NS = None
TRACE = False


def kernel(hidden_states, target_hidden, cos, sin, Wq, Wk, Wv, Wo):
    global LAST_EXEC_NS
    if "nc" not in _CACHE:
        _CACHE["nc"] = build_program()
    nc = _CACHE["nc"]
    in_maps = make_in_maps(
        hidden_states, target_hidden, cos, sin, Wq, Wk, Wv, Wo
    )
    res = run_bass_kernel_spmd(
        nc, in_maps, list(range(NCORES)), trace=TRACE
    )
    LAST_EXEC_NS = res.exec_time_ns
    out = np.zeros((QL, H), dtype=np.float32)
    for r in res.results:
        out += r["out"]
    return out.reshape(1, QL, H)


# revision 8
# speedup vs baseline: 1.0876x; 1.0876x over previous
"""DFlashAttention kernel for Trainium2, tensor-parallel across 8 NeuronCores.

Sharding: Megatron-style head parallelism. Core c owns KV head c and Q heads
4c..4c+3 (matches repeat_interleave grouping), i.e. Wq rows [512c, 512c+512),
Wk/Wv rows [128c, 128c+128), Wo columns [512c, 512c+512). Each core computes a
partial output [QL, H]; the host sums the 8 partials (row-parallel Wo).

v2 layout/pipeline notes:
  - activations/weights fed feature-major (host pre-transposes), bf16
  - Q/K kept d-major [HD, seq] f32r for scores^T = K^T(dxk-tile) @ Q(dxq)
  - scores matmul pairs write a 2-bank [128,2,512] PSUM tile; one ACT exp
    covers both halves (halves ACT instruction overhead)
  - softmax denominator: rowsum matmuls packed 4-wide into PE column groups
    via tile_position (4 concurrent M=1 matmuls ~ cost of one), partials at
    partitions 0/32/64/96 summed by a ones4 matmul; reciprocal on DVE;
    denominator broadcast along partitions via PE ones matmul
  - V kept k-major [seq, HD] (PE-transposed after d-major projection), bf16;
    PV accumulates attn^T = V^T @ P^T in PSUM over 32 k-tiles
  - Wo bf16, preloaded at kernel start; Wo matmuls of q-block n interleaved
    into the attention of q-block n+1 so the PE stays fed while ACT does exp
  - RoPE rotate-half swap via DVE partition-offset copies (no DMA);
    target_hidden loads issued on the ACT DMA queue to halve SP pressure
"""

import math
from contextlib import ExitStack

import ml_dtypes
import numpy as np

import concourse.bass as bass
import concourse.bacc as bacc
import concourse.mybir as mybir
import concourse.tile as tile
from concourse.bass_utils import run_bass_kernel_spmd

F32 = mybir.dt.float32
F32R = mybir.dt.float32r
BF16 = mybir.dt.bfloat16
AF = mybir.ActivationFunctionType
ALU = mybir.AluOpType

# Full-problem dims (hardcoded per spec)
B, QL, CTX, H = 1, 2048, 2048, 4096
NH, NKV, HD = 32, 8, 128
NCORES = 8
HPC = NH // NKV  # 4 q-heads per core (one KV head per core)


def build_program(ql=QL, ctx_len=CTX, h=H, trace_sim=False, phases="ABC", body_reps=1):
    """Build the per-core Bass program (SPMD: same program, per-core shards)."""
    s = ql + ctx_len          # total kv length
    et = h // 128             # e-tiles (contraction tiles for projections)
    kt = s // 128             # k-tiles in attention
    QC = 512                  # phase A position-chunk
    nch = ql // QC            # chunks (ctx assumed == ql)
    assert ctx_len == ql, "phase A chunking assumes ctx_len == ql"
    QB = 512                  # phase B q-block
    nqb = ql // QB
    scale = 1.0 / math.sqrt(HD)
    DQ = HPC * HD             # 512: per-core q-head dim
    hot = h // 512            # output-column chunks in Wo stage

    nc = bacc.Bacc("TRN2", target_bir_lowering=False, debug=False)

    def din(name, shape, dt_=F32):
        return nc.dram_tensor(name, shape, dt_, kind="ExternalInput").ap()

    hiddenT = din("hiddenT", [h, ql], BF16)      # hidden_states[0].T
    targetT = din("targetT", [h, ctx_len], BF16)  # target_hidden[0].T
    cosT = din("cosT", [HD, s])            # cos[0].T
    sinT = din("sinT", [HD, s])            # sign-folded sin[0].T
    wqT = din("wqT", [h, DQ], BF16)        # Wq[shard].T
    wkT = din("wkT", [h, HD], BF16)
    wvT = din("wvT", [h, HD], BF16)
    woT = din("woT", [DQ, h], BF16)        # Wo[:, shard].T
    onesb_d = din("ones_bf", [128, 1], BF16)
    onesb2_d = din("ones_bf2", [128, 128], BF16)
    ident_d = din("ident", [128, 128])
    out_d = nc.dram_tensor("out", [ql, h], F32, kind="ExternalOutput").ap()

    with tile.TileContext(nc, trace_sim=trace_sim) as tc, ExitStack() as ctx:
        persist = ctx.enter_context(tc.tile_pool(name="persist", bufs=1))

        qr_sb = persist.tile([128, HPC, ql], F32R, tag="qr")    # [d, h, q]
        kr_sb = persist.tile([128, s], F32R, tag="kr")          # [d, k]
        v_sb = persist.tile([128, kt, 128], BF16, tag="v")     # [k%128, ktile, d]
        wo_sb = persist.tile([128, HPC, h], BF16, tag="wo")    # [t%128, t//128, o]
        onesb_sb = persist.tile([128, 1], BF16, tag="onesb")
        onesb2_sb = persist.tile([128, 128], BF16, tag="onesb2")
        ident_sb = persist.tile([128, 128], F32, tag="ident")
        nc.sync.dma_start(onesb_sb[:], onesb_d[:])
        nc.sync.dma_start(onesb2_sb[:], onesb2_d[:])
        nc.sync.dma_start(ident_sb[:], ident_d[:])

        # ---------------- Phase A: projections + RoPE + V transpose ---------
        for _rep in range(body_reps):
          with (
              tc.tile_pool(name="wpool", bufs=1) as wpool,
              tc.tile_pool(name="apool", bufs=1) as apool,
              tc.tile_pool(name="psA", bufs=8, space=bass.MemorySpace.PSUM) as psA,
          ):
              wq_sb = wpool.tile([128, et, DQ], BF16, tag="wq")   # [e%128, etile, d]
              wk_sb = wpool.tile([128, et, HD], BF16, tag="wk")
              wv_sb = wpool.tile([128, et, HD], BF16, tag="wv")
              wq_r = wqT.rearrange("(e p) d -> p e d", p=128)
              wk_r = wkT.rearrange("(e p) d -> p e d", p=128)
              wv_r = wvT.rearrange("(e p) d -> p e d", p=128)
              wo_r = woT.rearrange("(t p) o -> p t o", p=128)

              def emit_weight_chunk(wc):
                  # interleaved into chunk 0's e-loop: first matmuls start early
                  sl = slice(wc * (et // 4), (wc + 1) * (et // 4))
                  nc.sync.dma_start(wk_sb[:, sl, :], wk_r[:, sl, :])
                  nc.sync.dma_start(wv_sb[:, sl, :], wv_r[:, sl, :])
                  nc.sync.dma_start(wq_sb[:, sl, :], wq_r[:, sl, :])

              def rope(ps_tile, cos_sl, sin_sl, dst, dve_raw=False,
                       act_swp=False):
                  # dst = ps*cos + rot_half(ps)*sin  (sin sign pre-folded)
                  raw = apool.tile([128, QC], F32, tag="rraw", bufs=3)
                  if dve_raw:
                      nc.vector.tensor_copy(raw[:], ps_tile[:])
                  else:
                      nc.scalar.copy(raw[:], ps_tile[:])
                  swp = apool.tile([128, QC], F32, tag="rswp", bufs=3)
                  if act_swp:
                      nc.scalar.copy(swp[0:64, :], raw[64:128, :])
                      nc.scalar.copy(swp[64:128, :], raw[0:64, :])
                  else:
                      nc.vector.tensor_copy(swp[0:64, :], raw[64:128, :])
                      nc.vector.tensor_copy(swp[64:128, :], raw[0:64, :])
                  t1 = apool.tile([128, QC], F32, tag="rt1", bufs=2)
                  nc.vector.tensor_tensor(t1[:], raw[:], cos_sl, ALU.mult)
                  t2 = apool.tile([128, QC], F32, tag="rt2", bufs=2)
                  nc.gpsimd.tensor_tensor(t2[:], swp[:], sin_sl, ALU.mult)
                  nc.vector.tensor_tensor(dst, t1[:], t2[:], ALU.add)

              for c in range(nch):
                  q0 = c * QC
                  cn = apool.tile([128, QC], F32, tag="cn", bufs=1)
                  sn = apool.tile([128, QC], F32, tag="sn", bufs=1)
                  cc = apool.tile([128, QC], F32, tag="cc", bufs=1)
                  sc = apool.tile([128, QC], F32, tag="sc", bufs=1)
                  nc.scalar.dma_start(cn[:], cosT[:, ctx_len + q0:ctx_len + q0 + QC])
                  nc.scalar.dma_start(sn[:], sinT[:, ctx_len + q0:ctx_len + q0 + QC])
                  nc.scalar.dma_start(cc[:], cosT[:, q0:q0 + QC])
                  nc.scalar.dma_start(sc[:], sinT[:, q0:q0 + QC])

                  psq = [
                      psA.tile([128, QC], F32, tag="ps", name=f"psq{i}")
                      for i in range(HPC)
                  ]
                  pskn = psA.tile([128, QC], F32, tag="ps")
                  pskc = psA.tile([128, QC], F32, tag="ps")
                  psvn = psA.tile([128, QC], F32, tag="ps")
                  psvc = psA.tile([128, QC], F32, tag="ps")

                  hid_r = hiddenT.rearrange("(b p) q -> p b q", p=128)
                  tgt_r = targetT.rearrange("(b p) q -> p b q", p=128)
                  for e0 in range(0, et, 2):
                      if c == 0 and e0 % (et // 4) == 0:
                          emit_weight_chunk(e0 // (et // 4))
                      hs = apool.tile([128, 2, QC], BF16, tag="hs", bufs=3)
                      nc.sync.dma_start(
                          hs[:], hid_r[:, e0:e0 + 2, q0:q0 + QC]
                      )
                      ts_ = apool.tile([128, 2, QC], BF16, tag="ts", bufs=3)
                      nc.scalar.dma_start(
                          ts_[:], tgt_r[:, e0:e0 + 2, q0:q0 + QC]
                      )
                      for u in (0, 1):
                          e = e0 + u
                          st = dict(start=(e == 0), stop=(e == et - 1))
                          for hh in range(HPC):
                              nc.tensor.matmul(
                                  psq[hh][:],
                                  wq_sb[:, e, hh * 128:hh * 128 + 128],
                                  hs[:, u, :],
                                  **st,
                              )
                          nc.tensor.matmul(pskn[:], wk_sb[:, e, :], hs[:, u, :], **st)
                          nc.tensor.matmul(psvn[:], wv_sb[:, e, :], hs[:, u, :], **st)
                          nc.tensor.matmul(pskc[:], wk_sb[:, e, :], ts_[:, u, :], **st)
                          nc.tensor.matmul(psvc[:], wv_sb[:, e, :], ts_[:, u, :], **st)

                  # RoPE + V drain. In the last chunk, finish kr/v first so
                  # phase B's first scores aren't gated on the Q ropes.
                  def drain_kv():
                      rope(pskn, cn[:], sn[:],
                           kr_sb[:, ctx_len + q0:ctx_len + q0 + QC], dve_raw=True)
                      rope(pskc, cc[:], sc[:], kr_sb[:, q0:q0 + QC])
                      for vsrc, kbase in ((psvc, q0), (psvn, ctx_len + q0)):
                          vd = apool.tile([128, QC], BF16, tag="vd", bufs=2)
                          nc.scalar.copy(vd[:], vsrc[:])
                          j0 = kbase // 128
                          nc.sync.dma_start_transpose(
                              v_sb[:, j0:j0 + QC // 128, :], vd[:]
                          )

                  def drain_q():
                      for hh in range(HPC):
                          rope(psq[hh], cn[:], sn[:], qr_sb[:, hh, q0:q0 + QC],
                               dve_raw=(hh % 2 == 1), act_swp=True)

                  if c == nch - 1:
                      drain_kv()
                      drain_q()
                  else:
                      drain_q()
                      drain_kv()

                  # wo chunk load on the ACT queue, spread across chunks
                  wsl = slice(c * (HPC // nch) if HPC >= nch else c,
                              (c + 1) * max(HPC // nch, 1))
                  nc.scalar.dma_start(wo_sb[:, wsl, :], wo_r[:, wsl, :])

          # ---------------- Phase B/C: attention + output projection ----------
          with (
              tc.tile_pool(name="bpool", bufs=1) as bpool,
              tc.tile_pool(name="psB", bufs=1, space=bass.MemorySpace.PSUM) as psB,
          ):
              ats = {}

              def emit_head(qb, hh, between=None):
                  qs0 = qb * QB
                  qsl = qr_sb[:, hh, qs0:qs0 + QB]
                  expst = bpool.tile([128, kt, QB], BF16, tag="expst", bufs=2)
                  psat = psB.tile([128, QB], F32, tag="psat", bufs=1)
                  # rowsum fold tree: f holds 16+8+4+2+1 partial tiles (bf16)
                  fold = bpool.tile([128, 31, QB], BF16, tag="fold", bufs=1)

                  def emit_scores_pair(g):
                      pss = psB.tile([128, 2, 512], F32, tag="pss", bufs=2)
                      for u in (0, 1):
                          j = 2 * g + u
                          nc.tensor.matmul(
                              pss[:, u, :],
                              kr_sb[:, j * 128:j * 128 + 128],
                              qsl,
                              start=True,
                              stop=True,
                          )
                      nc.scalar.activation(
                          expst[:, 2 * g:2 * g + 2, :], pss[:], AF.Exp, scale=scale
                      )

                  LOOK = 2
                  npair = kt // 2
                  for g in range(min(LOOK, npair)):
                      emit_scores_pair(g)
                  for g in range(npair):
                      if g + LOOK < npair:
                          emit_scores_pair(g + LOOK)
                      for u in (0, 1):
                          j = 2 * g + u
                          nc.tensor.matmul(
                              psat[:],
                              v_sb[:, j, :],
                              expst[:, j, :],
                              start=(j == 0),
                              stop=(j == kt - 1),
                          )
                      # level-1 fold of this exp pair (sum of its 2 tiles)
                      eng = nc.gpsimd if g % 2 == 0 else nc.vector
                      eng.tensor_tensor(
                          fold[:, g, :], expst[:, 2 * g, :],
                          expst[:, 2 * g + 1, :], ALU.add,
                      )
                      # higher tree nodes whose children just completed
                      gg = g
                      base_in, base_out, width = 0, 16, 8
                      while gg % 2 == 1 and width >= 1:
                          i = gg // 2
                          nc.vector.tensor_tensor(
                              fold[:, base_out + i, :],
                              fold[:, base_in + 2 * i, :],
                              fold[:, base_in + 2 * i + 1, :], ALU.add,
                          )
                          gg = i
                          base_in = base_out
                          base_out += width
                          width //= 2
                      if between is not None:
                          between(g)

                  # denominator: all-ones matmul reduces fold root over
                  # partitions AND broadcasts the sum to all 128 partitions
                  psd = psB.tile([128, QB], F32, tag="psrs", bufs=1)
                  nc.tensor.matmul(
                      psd[:], onesb2_sb[:], fold[:, 30, :], start=True, stop=True
                  )
                  recd = bpool.tile([128, QB], F32R, tag="recip", bufs=2)
                  with nc.allow_low_precision(
                      reason="f32r reciprocal feeds the normalize multiply"
                  ):
                      nc.vector.reciprocal(recd[:], psd[:])
                  at = bpool.tile([128, QB], BF16, tag="at", bufs=8)
                  nc.vector.tensor_tensor(at[:], psat[:], recd[:], ALU.mult)
                  ats[(qb, hh)] = at

              def emit_c_chain(qb, qs, oc):
                  qs0 = qb * QB
                  pso = psB.tile([128, 512], F32, tag="pso", bufs=2)
                  for t in range(HPC):
                      nc.tensor.matmul(
                          pso[:],
                          ats[(qb, t)][:, qs * 128:qs * 128 + 128],
                          wo_sb[:, t, oc * 512:oc * 512 + 512],
                          start=(t == 0),
                          stop=(t == HPC - 1),
                      )
                  ob = bpool.tile([128, 512], F32, tag="ob", bufs=3)
                  if (qs + oc) % 2 == 0:
                      nc.scalar.copy(ob[:], pso[:])
                  else:
                      nc.vector.tensor_copy(ob[:], pso[:])
                  nc.sync.dma_start(
                      out_d[qs0 + qs * 128:qs0 + qs * 128 + 128,
                            oc * 512:oc * 512 + 512],
                      ob[:],
                  )

              if "B" in phases:
                  for qb in range(nqb):
                      for hh in range(HPC):
                          if qb > 0 and "C" in phases:
                              # one Wo chain after every 2 score pairs: the
                              # pso-bank wait always coincides with ready
                              # attention matmuls
                              emit_head(qb, hh, between=lambda g, _q=qb - 1,
                                        _s=hh: emit_c_chain(_q, _s, g // 2)
                                        if g % 2 == 1 else None)
                          else:
                              emit_head(qb, hh)
                  if "C" in phases:
                      for qs in range(QB // 128):
                          for oc in range(hot):
                              emit_c_chain(nqb - 1, qs, oc)
    return _finish(nc)


def _finish(nc):
    nc.compile()
    return nc


def make_in_maps(hidden_states, target_hidden, cos, sin, Wq, Wk, Wv, Wo):
    hidden_states = np.asarray(hidden_states, dtype=np.float32)
    target_hidden = np.asarray(target_hidden, dtype=np.float32)
    cos = np.asarray(cos, dtype=np.float32)
    sin = np.asarray(sin, dtype=np.float32)
    Wq = np.asarray(Wq, dtype=np.float32)
    Wk = np.asarray(Wk, dtype=np.float32)
    Wv = np.asarray(Wv, dtype=np.float32)
    Wo = np.asarray(Wo, dtype=np.float32)

    bf16 = ml_dtypes.bfloat16
    hT = np.ascontiguousarray(hidden_states[0].T).astype(bf16)
    tT = np.ascontiguousarray(target_hidden[0].T).astype(bf16)
    cT = np.ascontiguousarray(cos[0].T)
    sT = np.ascontiguousarray(sin[0].T).copy()
    sT[:64, :] *= -1.0  # fold rotate_half sign: rot(x)*sin == swap(x)*sT
    ident = np.eye(128, dtype=np.float32)

    in_maps = []
    for c in range(NCORES):
        in_maps.append({
            "hiddenT": hT,
            "targetT": tT,
            "cosT": cT,
            "sinT": sT,
            "wqT": np.ascontiguousarray(Wq[512 * c:512 * c + 512, :].T).astype(bf16),
            "wkT": np.ascontiguousarray(Wk[128 * c:128 * c + 128, :].T).astype(bf16),
            "wvT": np.ascontiguousarray(Wv[128 * c:128 * c + 128, :].T).astype(bf16),
            "woT": np.ascontiguousarray(Wo[:, 512 * c:512 * c + 512].T).astype(bf16),
            "ones_bf": np.ones((128, 1), dtype=bf16),
            "ones_bf2": np.ones((128, 128), dtype=bf16),
            "ident": ident,
        })
    return in_maps


_CACHE = {}
LAST_EXEC_NS = None
TRACE = False


def kernel(hidden_states, target_hidden, cos, sin, Wq, Wk, Wv, Wo):
    global LAST_EXEC_NS
    if "nc" not in _CACHE:
        _CACHE["nc"] = build_program()
    nc = _CACHE["nc"]
    in_maps = make_in_maps(
        hidden_states, target_hidden, cos, sin, Wq, Wk, Wv, Wo
    )
    res = run_bass_kernel_spmd(
        nc, in_maps, list(range(NCORES)), trace=TRACE
    )
    LAST_EXEC_NS = res.exec_time_ns
    out = np.zeros((QL, H), dtype=np.float32)
    for r in res.results:
        out += r["out"]
    return out.reshape(1, QL, H)
